# revision 1
# baseline (speedup 1.0000x reference)
"""Multi-head self-attention Trainium2 kernel v2 (8 NeuronCores, SPMD).

Sharding: data-parallel over batch B=8 -> one batch element per core.

Single-core pipeline (bf16 matmuls, fp32 PSUM):
  qkvT = (x @ w_qkv)^T            q,k transposed; v natural+augmented
  sT_h[m,n] = k_h @ q_h^T         keys on partitions, queries free
  expT = exp(sT)                  scores in ~[-2,2]: no max subtraction
  outT_h = [v_h | 1]^T @ expT     ones column gives softmax denominator
  out_h = outT_h[:64] / outT_h[64]
  yT = w_proj^T @ outT + b_proj

v2 vs baseline:
  - software-pipelined supersteps: scores(c) + qk-proj(c+2) + AV(c-1)
    interleaved at matmul granularity so the PE never waits for exp
    (ACT engine) and ACT/DVE stay busy under PE's schedule
  - reciprocal broadcast via gpsimd partition_broadcast (SBUF->SBUF)
    instead of two DRAM DMA round-trips per divide
  - even heads' normalize writes oT directly (DVE); odd heads need the
    partition shift 0:64 -> 64:128, via one small SBUF->SBUF DMA
  - PSUM: 2x[128,1024] scores + 2x[128,512] qkv/proj + 2x[65,512] AV = 8 banks
  - q/k chunk tiles and exp tiles ring-pooled with lifetimes matched to
    the superstep schedule (fits 28MB SBUF with ~12KB/partition slack)
"""

from contextlib import ExitStack

import numpy as np
import ml_dtypes

import concourse.bass as bass
import concourse.mybir as mybir
import concourse.tile as tile
from concourse import bacc

BF16 = mybir.dt.bfloat16
F32 = mybir.dt.float32
P = 128  # SBUF partitions


def build_module(N=1024, D=1024, H=16, DK=64, reps=1):
    KC = D // P           # feature chunks (8)
    MC = N // P           # token chunks (8)
    FREE = 512            # moving free-dim per matmul (one PSUM bank fp32)
    NF = N // FREE        # 2
    assert H == 2 * KC

    nc = bacc.Bacc("TRN2", target_bir_lowering=False, debug=False)

    xT_d = nc.dram_tensor("xT", [D, N], BF16, kind="ExternalInput").ap()
    wq_d = nc.dram_tensor("wq", [D, D], BF16, kind="ExternalInput").ap()
    wk_d = nc.dram_tensor("wk", [D, D], BF16, kind="ExternalInput").ap()
    wv_d = nc.dram_tensor("wv", [D, D], BF16, kind="ExternalInput").ap()
    wp_d = nc.dram_tensor("wp", [D, D], BF16, kind="ExternalInput").ap()
    bq_d = nc.dram_tensor("bq", [P, KC], F32, kind="ExternalInput").ap()
    bk_d = nc.dram_tensor("bk", [P, KC], F32, kind="ExternalInput").ap()
    bvb_d = nc.dram_tensor("bvb", [P, D], BF16, kind="ExternalInput").ap()
    bp_d = nc.dram_tensor("bp", [P, KC], F32, kind="ExternalInput").ap()
    yT_d = nc.dram_tensor("yT", [D, N], F32, kind="ExternalOutput").ap()

    xT_v = xT_d.rearrange("(c p) n -> p c n", p=P)
    wq_v = wq_d.rearrange("(c p) n -> p c n", p=P)
    wk_v = wk_d.rearrange("(c p) n -> p c n", p=P)
    wv_v = wv_d.rearrange("(c p) n -> p c n", p=P)
    wp_v = wp_d.rearrange("(c p) n -> p c n", p=P)
    yT_v = yT_d.rearrange("(c p) n -> p c n", p=P)

    with tile.TileContext(nc) as tc, ExitStack() as ctx:
        consts = ctx.enter_context(tc.tile_pool(name="consts", bufs=1))
        perst = ctx.enter_context(tc.tile_pool(name="perst", bufs=1))
        psS = ctx.enter_context(tc.tile_pool(name="psS", bufs=2, space="PSUM"))
        psQ = ctx.enter_context(tc.tile_pool(name="psQ", bufs=2, space="PSUM"))
        psPO = ctx.enter_context(tc.tile_pool(name="psPO", bufs=2, space="PSUM"))
        qp = ctx.enter_context(tc.tile_pool(name="qp", bufs=4))
        kp = ctx.enter_context(tc.tile_pool(name="kp", bufs=4))
        exA_p = ctx.enter_context(tc.tile_pool(name="exA", bufs=12))
        exB_p = ctx.enter_context(tc.tile_pool(name="exB", bufs=14))
        misc_p = ctx.enter_context(tc.tile_pool(name="misc", bufs=2))
        yst_p = ctx.enter_context(tc.tile_pool(name="ystp", bufs=3))

        wq_sb = consts.tile([P, KC, D], BF16)
        wk_sb = consts.tile([P, KC, D], BF16)
        wv_sb = consts.tile([P, KC, D], BF16)
        wp_sb = consts.tile([P, KC, D], BF16)
        bq_sb = consts.tile([P, KC], F32)
        bk_sb = consts.tile([P, KC], F32)
        bp_sb = consts.tile([P, KC], F32)
        bvb_sb = consts.tile([P, D], BF16)

        xT_sb = perst.tile([P, KC, N], BF16)
        vaug_sb = perst.tile([P, MC, H, DK + 1], BF16)
        oT_sb = perst.tile([P, KC, N], BF16)
        nc.vector.memset(vaug_sb[:, :, :, DK : DK + 1], 1.0)

        for _rep in range(reps):
            # ---- input DMA (weights once; x re-loaded per rep) ----
            for c in range(KC):
                nc.sync.dma_start(out=xT_sb[:, c, :], in_=xT_v[:, c, :])
                if _rep == 0:
                    nc.sync.dma_start(out=wv_sb[:, c, :], in_=wv_v[:, c, :])
            if _rep == 0:
                nc.sync.dma_start(out=bvb_sb, in_=bvb_d)
                nc.sync.dma_start(out=bq_sb, in_=bq_d)
                nc.sync.dma_start(out=bk_sb, in_=bk_d)
                for c in range(KC):
                    nc.sync.dma_start(out=wq_sb[:, c, :], in_=wq_v[:, c, :])
                for c in range(KC):
                    nc.sync.dma_start(out=wk_sb[:, c, :], in_=wk_v[:, c, :])
                for c in range(KC):
                    nc.sync.dma_start(out=wp_sb[:, c, :], in_=wp_v[:, c, :])
                nc.sync.dma_start(out=bp_sb, in_=bp_d)

            # ---- v projection (natural layout into vaug) ----
            for m in range(MC):
                for f in range(NF):
                    ps = psQ.tile([P, FREE], F32, tag="psQ", name="ps_v")
                    for k in range(KC):
                        nc.tensor.matmul(
                            ps,
                            lhsT=xT_sb[:, k, m * P : (m + 1) * P],
                            rhs=wv_sb[:, k, f * FREE : (f + 1) * FREE],
                            start=(k == 0),
                            stop=(k == KC - 1),
                            skip_group_check=True,
                        )
                    nc.vector.tensor_add(
                        out=vaug_sb[:, m, 8 * f : 8 * (f + 1), 0:DK],
                        in0=ps,
                        in1=bvb_sb[:, f * FREE : (f + 1) * FREE],
                    )

            # ---- q/k projection (chunk tiles from ring pools) ----
            qt, kt = {}, {}

            def qk_mms(c):
                """Yield thunks: 32 matmuls + 4 drains for q,k of chunk c."""
                qt[c] = qp.tile([P, N], BF16, tag="q", name="qt")
                kt[c] = kp.tile([P, N], BF16, tag="k", name="kt")
                for dst, w_sb, b_sb in (
                    (qt[c], wq_sb, bq_sb),
                    (kt[c], wk_sb, bk_sb),
                ):
                    for f in range(NF):
                        ps = psQ.tile([P, FREE], F32, tag="psQ", name="ps_qk")
                        for k in range(KC):

                            def mm(ps=ps, w_sb=w_sb, c=c, f=f, k=k):
                                nc.tensor.matmul(
                                    ps,
                                    lhsT=w_sb[:, k, c * P : (c + 1) * P],
                                    rhs=xT_sb[:, k, f * FREE : (f + 1) * FREE],
                                    start=(k == 0),
                                    stop=(k == KC - 1),
                                    skip_group_check=True,
                                )

                            yield mm

                        def drain(ps=ps, dst=dst, b_sb=b_sb, c=c, f=f):
                            nc.vector.tensor_scalar_add(
                                out=dst[:, f * FREE : (f + 1) * FREE],
                                in0=ps,
                                scalar1=b_sb[:, c : c + 1],
                            )

                        yield drain

            def emit_qk(c):
                for th in qk_mms(c):
                    th()

            emit_qk(0)
            emit_qk(1)

            qk_stream = []
            for c in range(2, KC):
                qk_stream.extend(qk_mms(c))
            qk_stream.reverse()  # pop() takes from the front

            ex_tiles = {}

            def emit_scores(c, j):
                for hl, rows, tp, exp in (
                    (0, slice(0, DK), (0, 0), exA_p),
                    (1, slice(DK, P), (DK, 0), exB_p),
                ):
                    s = psS.tile([P, N], F32, tag="psS", name="s")
                    for f in range(NF):
                        nc.tensor.matmul(
                            s[:, f * FREE : (f + 1) * FREE],
                            lhsT=kt[c][rows, j * P : (j + 1) * P],
                            rhs=qt[c][rows, f * FREE : (f + 1) * FREE],
                            start=True,
                            stop=True,
                            tile_position=tp,
                            skip_group_check=True,
                        )
                    ex = exp.tile([P, N], BF16, tag="ex", name="ex")
                    nc.scalar.activation(
                        out=ex, in_=s, func=mybir.ActivationFunctionType.Exp
                    )
                    ex_tiles[(c, j, hl)] = ex

            av_state = {}

            def emit_av(c, j, tl_order=(0, 1, 2, 3)):
                """po tile tl accumulates key chunks over two j-steps, then
                drains: reciprocal (DVE), partition broadcast (gpsimd),
                normalize-multiply (DVE), odd heads bounce via small DMA."""
                tl = tl_order[j // 2]
                hl, f = tl // 2, tl % 2
                h = c * 2 + hl
                if j % 2 == 0:
                    av_state[tl] = psPO.tile(
                        [DK + 1, FREE], F32, tag="psPO", name="po"
                    )
                po = av_state[tl]
                for jj in range(4 * (j % 2), 4 * (j % 2) + 4):
                    nc.tensor.matmul(
                        po,
                        lhsT=vaug_sb[:, jj, h, :],
                        rhs=ex_tiles[(c, jj, hl)][:, f * FREE : (f + 1) * FREE],
                        start=(jj == 0),
                        stop=(jj == MC - 1),
                        skip_group_check=True,
                    )
                if j % 2 == 1:
                    rc = misc_p.tile([1, FREE], F32, tag="rc", name="rc")
                    nc.vector.reciprocal(out=rc, in_=po[DK : DK + 1, :])
                    rcb = misc_p.tile([DK, FREE], F32, tag="rcb", name="rcb")
                    nc.gpsimd.partition_broadcast(rcb, rc)
                    fs = slice(f * FREE, (f + 1) * FREE)
                    if hl == 0:
                        nc.vector.tensor_mul(
                            out=oT_sb[0:DK, c, fs], in0=po[0:DK, :], in1=rcb
                        )
                    else:
                        tmpo = misc_p.tile([DK, FREE], BF16, tag="tmpo", name="tmpo")
                        nc.vector.tensor_mul(out=tmpo, in0=po[0:DK, :], in1=rcb)
                        nc.sync.dma_start(out=oT_sb[DK:P, c, fs], in_=tmpo)

            # ---- attention supersteps: scores(c) + qk(c+2) + AV(c-1) ----
            for c in range(KC):
                for j in range(MC):
                    emit_scores(c, j)
                    for _ in range(4):
                        if qk_stream:
                            qk_stream.pop()()
                    if c > 0:
                        emit_av(c - 1, j)
            for j in range(MC):  # AV tail: f0 tiles first so proj(0)'s late
                # f1 reads overlap the final bounce DMA instead of waiting
                emit_av(KC - 1, j, tl_order=(0, 2, 1, 3))

            # ---- output projection ----
            # proj(c) runs both f-halves through one 2-bank tile from the
            # scores pool (idle now; same-tag ring keeps WAR tracking on the
            # proven same-pool path) -> drain-ring stalls vanish without
            # touching the 8-bank PSUM budget
            for c in range(KC):
                ps = psS.tile([P, N], F32, tag="psS", name="ps_proj")
                for f in range(NF):
                    for k in range(KC):
                        nc.tensor.matmul(
                            ps[:, f * FREE : (f + 1) * FREE],
                            lhsT=wp_sb[:, k, c * P : (c + 1) * P],
                            rhs=oT_sb[:, k, f * FREE : (f + 1) * FREE],
                            start=(k == 0),
                            stop=(k == KC - 1),
                            skip_group_check=True,
                        )
                for f in range(NF):
                    yst = yst_p.tile([P, FREE], F32, tag="yst", name="yst")
                    nc.vector.tensor_scalar_add(
                        out=yst,
                        in0=ps[:, f * FREE : (f + 1) * FREE],
                        scalar1=bp_sb[:, c : c + 1],
                    )
                    # last chunk drains on the idle ACT engine's queue so
                    # the final output DMAs parallel sync's queue backlog
                    eng = nc.scalar if c == KC - 1 else nc.sync
                    eng.dma_start(
                        out=yT_v[:, c, f * FREE : (f + 1) * FREE], in_=yst
                    )

    nc.compile()
    return nc


def make_in_maps(x, w_qkv, b_qkv, w_proj, b_proj, N=1024, D=1024, H=16, DK=64):
    """Host-side prep: shard over batch, fold scale, transpose x, cast bf16."""
    bf = ml_dtypes.bfloat16
    KC = D // P
    scale = np.float32(1.0 / np.sqrt(DK))
    wq = np.ascontiguousarray((w_qkv[:, :D] * scale)).astype(bf)
    wk = np.ascontiguousarray(w_qkv[:, D : 2 * D]).astype(bf)
    wv = np.ascontiguousarray(w_qkv[:, 2 * D :]).astype(bf)
    wp = np.ascontiguousarray(w_proj).astype(bf)
    bq = np.ascontiguousarray((b_qkv[:D] * scale).reshape(KC, P).T).astype(np.float32)
    bk = np.ascontiguousarray(b_qkv[D : 2 * D].reshape(KC, P).T).astype(np.float32)
    bvb = np.ascontiguousarray(np.broadcast_to(b_qkv[2 * D :], (P, D))).astype(bf)
    bp = np.ascontiguousarray(b_proj.reshape(KC, P).T).astype(np.float32)
    in_maps = []
    for b in range(x.shape[0]):
        xT = np.ascontiguousarray(x[b].T).astype(bf)
        in_maps.append(
            dict(xT=xT, wq=wq, wk=wk, wv=wv, wp=wp, bq=bq, bk=bk, bvb=bvb, bp=bp)
        )
    return in_maps


_module_cache = {}


def kernel(x, w_qkv, b_qkv, w_proj, b_proj):
    from concourse.bass_utils import run_bass_kernel_spmd

    x = np.asarray(x)
    B = x.shape[0]
    if "nc" not in _module_cache:
        _module_cache["nc"] = build_module()
    nc = _module_cache["nc"]
    in_maps = make_in_maps(
        x, np.asarray(w_qkv), np.asarray(b_qkv), np.asarray(w_proj), np.asarray(b_proj)
    )
    res = run_bass_kernel_spmd(nc, in_maps, core_ids=list(range(B)))
    out = np.stack([np.asarray(r["yT"]).T for r in res.results], axis=0)
    return np.ascontiguousarray(out.astype(np.float32))



# revision 26
# speedup vs baseline: 643.5943x; 643.5943x over previous
"""Multi-head self-attention Trainium2 kernel v3 (8 NeuronCores, SPMD).

Sharding: data-parallel over batch B=8 -> one batch element per core.

Single-core pipeline (bf16 matmuls, fp32 PSUM):
  qkvT = (x @ w_qkv)^T            q,k transposed; v natural+augmented
  sT_h[m,n] = k_h @ q_h^T         keys on partitions, queries free
  expT = exp(sT)                  scores in ~[-2,2]: no max subtraction
  outT_h = [v_h | 1]^T @ expT     ones column gives softmax denominator
  out_h = outT_h[:64] / outT_h[64]
  yT = w_proj^T @ outT + b_proj

v3 vs v2: row-group ping-pong for the tensor engine.
  Every 128-contraction matmul is split into two 64-row halves.  The
  top halves accumulate in one PSUM bank at tile_position (0,0), the
  bottom halves in a second bank at (64,0) (walrus requires each
  accumulation group to keep a single tile position equal to the
  stationary tensor's base partition), and a fused DVE op merges the
  two partial banks (+bias) at drain.  Emitted as strict T,B,T,B
  streams, the two members of each slot run CONCURRENTLY on the two
  64-row halves of the PE array (per-subarray row tiling), and every
  LDWEIGHTS hides under the opposite half's in-flight matmul - the
  ~107ns exposed weight-load per full-array matmul of v2 disappears
  and the scores hl0/hl1 pairs genuinely overlap.
  - accumulation brackets (start/stop) are derived from per-group
    emission counters, so slot order is free
  - q/k projection chunks 0-3 move into the DMA-gated prologue;
    chunks 4-7 stream through the supersteps (lag 2)
  - v-projection woven into superstep c=0 so ACT's exp stream (131us)
    fits inside the superstep span instead of starting 40us late
  - ~4us of warm-up matmuls on a zeroed scratch tile hold the PE HAM
    clock-gate at 8/8 while the first input DMAs land
  - proj chunk 0 accumulates inside the AV(7) tail (k-chunk 7 last)
  - PSUM: psS 2x[128,1024] scores/proj/warm + psQ 2x[128,512] qk/vproj
    + psPO 2x[65|128,512] AV/vproj = 8 banks
"""

from contextlib import ExitStack

import numpy as np
import ml_dtypes

import concourse.bass as bass
import concourse.mybir as mybir
import concourse.tile as tile
from concourse import bacc

BF16 = mybir.dt.bfloat16
F32 = mybir.dt.float32
P = 128  # SBUF partitions
HP = 64  # half-partition (row-group granularity)


class Grp:
    """PSUM accumulation-group bracket: start on the first emitted half,
    stop on the n-th.  Makes bracket placement order-independent."""

    def __init__(self, n):
        self.n = n
        self.i = 0

    def flags(self):
        s, e = self.i == 0, self.i == self.n - 1
        self.i += 1
        assert self.i <= self.n
        return s, e


def build_module(N=1024, D=1024, H=16, DK=64, reps=1, warmup=20):
    KC = D // P           # feature chunks (8)
    MC = N // P           # token chunks (8)
    FREE = 512            # moving free-dim per matmul (one PSUM bank fp32)
    NF = N // FREE        # 2
    assert H == 2 * KC and DK == HP

    nc = bacc.Bacc("TRN2", target_bir_lowering=False, debug=False)

    xT_d = nc.dram_tensor("xT", [D, N], BF16, kind="ExternalInput").ap()
    wq_d = nc.dram_tensor("wq", [D, D], BF16, kind="ExternalInput").ap()
    wk_d = nc.dram_tensor("wk", [D, D], BF16, kind="ExternalInput").ap()
    wv_d = nc.dram_tensor("wv", [D, D], BF16, kind="ExternalInput").ap()
    wp_d = nc.dram_tensor("wp", [D, D], BF16, kind="ExternalInput").ap()
    bq_d = nc.dram_tensor("bq", [P, KC], F32, kind="ExternalInput").ap()
    bk_d = nc.dram_tensor("bk", [P, KC], F32, kind="ExternalInput").ap()
    bvb_d = nc.dram_tensor("bvb", [P, D], BF16, kind="ExternalInput").ap()
    bp_d = nc.dram_tensor("bp", [P, KC], F32, kind="ExternalInput").ap()
    yT_d = nc.dram_tensor("yT", [D, N], F32, kind="ExternalOutput").ap()

    xT_v = xT_d.rearrange("(c p) n -> p c n", p=P)
    wq_v = wq_d.rearrange("(c p) n -> p c n", p=P)
    wk_v = wk_d.rearrange("(c p) n -> p c n", p=P)
    wv_v = wv_d.rearrange("(c p) n -> p c n", p=P)
    wp_v = wp_d.rearrange("(c p) n -> p c n", p=P)
    yT_v = yT_d.rearrange("(c p) n -> p c n", p=P)

    with tile.TileContext(nc) as tc, ExitStack() as ctx:
        consts = ctx.enter_context(tc.tile_pool(name="consts", bufs=1))
        perst = ctx.enter_context(tc.tile_pool(name="perst", bufs=1))
        psS = ctx.enter_context(tc.tile_pool(name="psS", bufs=2, space="PSUM"))
        psQ = ctx.enter_context(tc.tile_pool(name="psQ", bufs=2, space="PSUM"))
        psPO = ctx.enter_context(tc.tile_pool(name="psPO", bufs=2, space="PSUM"))
        qp = ctx.enter_context(tc.tile_pool(name="qp", bufs=4))
        kp = ctx.enter_context(tc.tile_pool(name="kp", bufs=4))
        exA_p = ctx.enter_context(tc.tile_pool(name="exA", bufs=14))
        exB_p = ctx.enter_context(tc.tile_pool(name="exB", bufs=14))
        misc_p = ctx.enter_context(tc.tile_pool(name="misc", bufs=2))
        mrg_p = ctx.enter_context(tc.tile_pool(name="mrg", bufs=3))
        yst_p = ctx.enter_context(tc.tile_pool(name="ystp", bufs=3))

        wq_sb = consts.tile([P, KC, D], BF16)
        wk_sb = consts.tile([P, KC, D], BF16)
        wv_sb = consts.tile([P, KC, D], BF16)
        wp_sb = consts.tile([P, KC, D], BF16)
        bq_sb = consts.tile([P, KC], F32)
        bk_sb = consts.tile([P, KC], F32)
        bp_sb = consts.tile([P, KC], F32)
        bvb_sb = consts.tile([P, D], BF16)
        warm_sb = consts.tile([P, FREE], BF16)

        xT_sb = perst.tile([P, KC, N], BF16)
        vaug_sb = perst.tile([P, MC, H, DK + 1], BF16)
        oT_sb = perst.tile([P, KC, N], BF16)
        nc.vector.memset(vaug_sb[:, :, :, DK : DK + 1], 1.0)
        nc.vector.memset(warm_sb, 0.0)

        ROWS = (slice(0, HP), slice(HP, P))
        TPOS = ((0, 0), (HP, 0))

        def half(out, lhsT_fn, rhs_fn, r, grp):
            """One 64-row half-matmul at row-group r (0=top)."""
            start, stop = grp.flags()
            nc.tensor.matmul(
                out,
                lhsT=lhsT_fn(ROWS[r]),
                rhs=rhs_fn(ROWS[r]),
                start=start,
                stop=stop,
                tile_position=TPOS[r],
                skip_group_check=True,
            )

        def pair_tile_slots(pX, pY, lhs_fn, rhs_fn, nk=KC):
            """One logical output accumulated over nk contraction chunks:
            top-half group in bank pX, bottom-half group in bank pY.
            Yields nk slot closures; each runs (T_k || B_k)."""
            gX, gY = Grp(nk), Grp(nk)
            for k in range(nk):
                def slot(k=k):
                    half(pX, lhs_fn(k), rhs_fn(k), 0, gX)
                    half(pY, lhs_fn(k), rhs_fn(k), 1, gY)

                yield slot

        def run(slots):
            for s in slots:
                s()

        for _rep in range(reps):
            # ---- input DMA (weights once; x re-loaded per rep) ----
            # ordered by first use: (x,wq,wk) per chunk for the prologue,
            # then wv (vproj in superstep c=0), then the late tensors
            for c in range(KC):
                nc.sync.dma_start(out=xT_sb[:, c, :], in_=xT_v[:, c, :])
                if _rep == 0:
                    nc.sync.dma_start(out=wq_sb[:, c, :], in_=wq_v[:, c, :])
                    nc.sync.dma_start(out=wk_sb[:, c, :], in_=wk_v[:, c, :])
            if _rep == 0:
                nc.sync.dma_start(out=bq_sb, in_=bq_d)
                nc.sync.dma_start(out=bk_sb, in_=bk_d)
                for c in range(KC):
                    nc.sync.dma_start(out=wv_sb[:, c, :], in_=wv_v[:, c, :])
                nc.sync.dma_start(out=bvb_sb, in_=bvb_d)
                for c in range(KC):
                    nc.sync.dma_start(out=wp_sb[:, c, :], in_=wp_v[:, c, :])
                nc.sync.dma_start(out=bp_sb, in_=bp_d)

            # ---- HAM warm-up: dummy matmuls on zeroed scratch while the
            # first x/wq/wk chunks stream in (rep 0 only) ----
            if _rep == 0:
                for _w in range(warmup):
                    ps = psS.tile([P, N], F32, tag="psS", name="warm")
                    nc.tensor.matmul(
                        ps[:, 0:FREE],
                        lhsT=warm_sb[:, 0:P],
                        rhs=warm_sb,
                        start=True,
                        stop=True,
                        skip_group_check=True,
                    )

            # ---- q/k projection: one logical tile = (dst, chunk c, f) ----
            qt, kt = {}, {}

            def qk_tile(c, f, w_sb, b_sb, dst, pool, tag):
                pX = pool.tile([P, FREE], F32, tag=tag, name="qkX")
                pY = pool.tile([P, FREE], F32, tag=tag, name="qkY")
                fs = slice(f * FREE, (f + 1) * FREE)
                slots = list(
                    pair_tile_slots(
                        pX,
                        pY,
                        lambda k: lambda rows: w_sb[rows, k, c * P : (c + 1) * P],
                        lambda k: lambda rows: xT_sb[rows, k, fs],
                    )
                )

                def drain():
                    # walrus: a DVE op may read only ONE input from PSUM,
                    # so stage bank Y (+bias) through SBUF, then add bank X
                    tmpS = mrg_p.tile([P, FREE], BF16, tag="mrg", name="mrg")
                    nc.vector.tensor_scalar_add(
                        out=tmpS, in0=pY, scalar1=b_sb[:, c : c + 1]
                    )
                    nc.vector.tensor_add(out=dst[:, fs], in0=pX, in1=tmpS)

                return slots, drain

            def new_qkt(c):
                qt[c] = qp.tile([P, N], BF16, tag="q", name="qt")
                kt[c] = kp.tile([P, N], BF16, tag="k", name="kt")

            # prologue: chunks 0..3; f0 tiles on psQ banks, f1 on psPO
            def emit_prologue_chunk(c):
                new_qkt(c)
                for w_sb, b_sb, dst in (
                    (wq_sb, bq_sb, qt[c]),
                    (wk_sb, bk_sb, kt[c]),
                ):
                    s0, d0 = qk_tile(c, 0, w_sb, b_sb, dst, psQ, "psQ")
                    s1, d1 = qk_tile(c, 1, w_sb, b_sb, dst, psPO, "psPO")
                    for a, b in zip(s0, s1):
                        a()
                        b()
                    d0()
                    d1()

            # superstep stream: chunk c split over per-step slot lists; one
            # logical (dst, f) tile spans 8/nsteps steps (psQ banks)
            def qk_stream(c, nsteps=8):
                new_qkt(c)
                per = 32 // nsteps  # slots per step
                specs = [
                    (w, b, d, f)
                    for w, b, d in ((wq_sb, bq_sb, qt[c]), (wk_sb, bk_sb, kt[c]))
                    for f in range(NF)
                ]
                for w_sb, b_sb, dst, f in specs:
                    slots, drain = qk_tile(c, f, w_sb, b_sb, dst, psQ, "psQ")
                    for lo in range(0, 8, per):
                        yield slots[lo : lo + per], (
                            drain if lo + per >= 8 else None
                        )

            # ---- v projection: logical tile (m, f); f0 on psQ banks,
            # f1 on psPO (both free during superstep c=0) ----
            def vproj_tile(m, f):
                pool = psQ if f == 0 else psPO
                tag = "psQ" if f == 0 else "psPO"
                pX = pool.tile([P, FREE], F32, tag=tag, name="vX")
                pY = pool.tile([P, FREE], F32, tag=tag, name="vY")
                fs = slice(f * FREE, (f + 1) * FREE)
                slots = list(
                    pair_tile_slots(
                        pX,
                        pY,
                        lambda k: lambda rows: xT_sb[rows, k, m * P : (m + 1) * P],
                        lambda k: lambda rows: wv_sb[rows, k, fs],
                    )
                )
                vsl = vaug_sb[:, m, 8 * f : 8 * (f + 1), 0:DK]

                def drain():
                    tmpS = mrg_p.tile([P, FREE], BF16, tag="mrg", name="mrg")
                    nc.vector.tensor_add(out=tmpS, in0=pY, in1=bvb_sb[:, fs])
                    nc.vector.tensor_add(out=vsl, in0=pX, in1=tmpS)

                return slots, drain

            # ---- scores + exp for one (c, j): 2 paired slots ----
            ex_tiles = {}

            def scores_step(c, j):
                s0 = psS.tile([P, N], F32, tag="psS", name="s0")
                s1 = psS.tile([P, N], F32, tag="psS", name="s1")
                ex0 = exA_p.tile([P, N], BF16, tag="ex", name="ex")
                ex1 = exB_p.tile([P, N], BF16, tag="ex", name="ex")
                ex_tiles[(c, j, 0)] = ex0
                ex_tiles[(c, j, 1)] = ex1

                def mk(f):
                    def slot():
                        for r, s in ((0, s0), (1, s1)):
                            nc.tensor.matmul(
                                s[:, f * FREE : (f + 1) * FREE],
                                lhsT=kt[c][ROWS[r], j * P : (j + 1) * P],
                                rhs=qt[c][ROWS[r], f * FREE : (f + 1) * FREE],
                                start=True,
                                stop=True,
                                tile_position=TPOS[r],
                                skip_group_check=True,
                            )

                    return slot

                def mkexp(s, ex):
                    def runx():
                        nc.scalar.activation(
                            out=ex, in_=s, func=mybir.ActivationFunctionType.Exp
                        )

                    return runx

                return [mk(0), mk(1)], mkexp(s0, ex0), mkexp(s1, ex1)

            # ---- AV: logical tile (c, hl, f) = [65,512] x 2 psPO banks;
            # 8 slots over jj; drain merges, normalizes, writes oT ----
            # ---- AV: v2-style full-array accumulation, one [65,512] bank
            # per (hl, f) tile, 4 matmuls per step over 2 j-steps.  (BIRSim
            # rejects mixed-tile-position accumulation groups and DVE can't
            # read two PSUM banks, so AV keeps the full-contraction form;
            # its ~54ns/matmul weight loads stay exposed.) ----
            av_state = {}

            def av_step(c, j):
                """tl order (0,2,1,3): both f0 tiles finish by step 3 so
                the tail's proj weave sees oT chunk 7 f0 early."""
                tl = (0, 2, 1, 3)[j // 2]
                hl, f = tl // 2, tl % 2
                h = c * 2 + hl
                if j % 2 == 0:
                    av_state[tl] = psPO.tile(
                        [DK + 1, FREE], F32, tag="psPO", name="po"
                    )
                po = av_state[tl]
                mms = []
                for jj in range(4 * (j % 2), 4 * (j % 2) + 4):
                    def mm(jj=jj, po=po, hl=hl, f=f, h=h):
                        nc.tensor.matmul(
                            po,
                            lhsT=vaug_sb[:, jj, h, :],
                            rhs=ex_tiles[(c, jj, hl)][
                                :, f * FREE : (f + 1) * FREE
                            ],
                            start=(jj == 0),
                            stop=(jj == MC - 1),
                            skip_group_check=True,
                        )

                    mms.append(mm)

                def post():
                    if j % 2 == 1:
                        drain_av(c, tl)

                return mms, post

            def drain_av(c, tl):
                """reciprocal (DVE) -> partition broadcast (gpsimd) ->
                normalize-multiply; odd heads bounce via small DMA."""
                hl, f = tl // 2, tl % 2
                po = av_state[tl]
                rc = misc_p.tile([1, FREE], F32, tag="rc", name="rc")
                nc.vector.reciprocal(out=rc, in_=po[DK : DK + 1, :])
                rcb = misc_p.tile([DK, FREE], F32, tag="rcb", name="rcb")
                nc.gpsimd.partition_broadcast(rcb, rc)
                fs = slice(f * FREE, (f + 1) * FREE)
                if hl == 0:
                    nc.vector.tensor_mul(
                        out=oT_sb[0:DK, c, fs], in0=po[0:DK, :], in1=rcb
                    )
                else:
                    tmpo = misc_p.tile([DK, FREE], BF16, tag="tmpo", name="tmpo")
                    nc.vector.tensor_mul(out=tmpo, in0=po[0:DK, :], in1=rcb)
                    nc.sync.dma_start(out=oT_sb[DK:P, c, fs], in_=tmpo)

            # ---- output projection: logical tile (c, f) lives in the two
            # banks of one [128,1024] psS tile; k-chunk 7 deferrable ----
            def proj_tile(c, f):
                ps = psS.tile([P, N], F32, tag="psS", name="ps_proj")
                pX, pY = ps[:, 0:FREE], ps[:, FREE:N]
                fs = slice(f * FREE, (f + 1) * FREE)
                slots = list(
                    pair_tile_slots(
                        pX,
                        pY,
                        lambda k: lambda rows: wp_sb[rows, k, c * P : (c + 1) * P],
                        lambda k: lambda rows: oT_sb[rows, k, fs],
                    )
                )

                def drain():
                    tmpS = mrg_p.tile([P, FREE], BF16, tag="mrg", name="mrg")
                    nc.vector.tensor_scalar_add(
                        out=tmpS, in0=pY, scalar1=bp_sb[:, c : c + 1]
                    )
                    yst = yst_p.tile([P, FREE], F32, tag="yst", name="yst")
                    nc.vector.tensor_add(out=yst, in0=pX, in1=tmpS)
                    # last chunk drains on the idle ACT engine's queue so
                    # the final output DMAs parallel sync's queue backlog
                    eng = nc.scalar if c == KC - 1 else nc.sync
                    eng.dma_start(out=yT_v[:, c, fs], in_=yst)

                return slots, drain

            # ================= emission =================
            # prologue: qk chunks 0-3 (DMA-paced; warm-up dummies cover it)
            for c in range(4):
                emit_prologue_chunk(c)

            # supersteps c=0..7: scores(c) + qk stream (chunks 4,5 full
            # rate at c=2,3; chunks 6,7 half rate over c=4..7 to keep the
            # ACT-bound late stages fed with PE work) + vproj at c=0 +
            # AV(c-1) at c>=1
            qk_gen = None
            for c in range(KC):
                if c == 2:
                    qk_gen = qk_stream(4, 8)
                elif c == 3:
                    qk_gen = qk_stream(5, 8)
                elif c == 4:
                    qk_gen = qk_stream(6, 16)
                elif c == 6:
                    qk_gen = qk_stream(7, 8)
                for j in range(MC):
                    sc, exp0, exp1 = scores_step(c, j)
                    qks, qkpost = next(qk_gen, ([], None)) if c >= 2 else ([], None)
                    avmm, avpost = av_step(c - 1, j) if c >= 1 else ([], None)
                    # interleave: scores early (their exps feed next-stage
                    # AV), qk/av spread between
                    sc[0]()
                    for s in qks[0:2]:
                        s()
                    for s in avmm[0:2]:
                        s()
                    sc[1]()
                    exp0()
                    exp1()
                    for s in qks[2:4]:
                        s()
                    for s in avmm[2:4]:
                        s()
                    if qkpost:
                        qkpost()
                    if avpost:
                        avpost()
                    if c == 0:
                        vs0, vd0 = vproj_tile(j, 0)
                        vs1, vd1 = vproj_tile(j, 1)
                        for a, b in zip(vs0, vs1):
                            a()
                            b()
                        vd0()
                        vd1()

            # AV tail for chunk 7, woven with proj chunk 0 (k<7 slots only;
            # chunk-7 oT lands at tail steps 3 (f0) and 7 (f1))
            p00, d00 = proj_tile(0, 0)
            p01, d01 = proj_tile(0, 1)
            weave = [p00[k] for k in range(7)] + [p01[k] for k in range(7)]
            for j in range(MC):
                avmm, avpost = av_step(KC - 1, j)
                for m in avmm:
                    m()
                avpost()
                run(weave[2 * j : 2 * j + 2])
            run([p00[7], p01[7]])
            d00()
            d01()

            for c in range(1, KC):
                for f in range(NF):
                    slots, drain = proj_tile(c, f)
                    run(slots)
                    drain()

    nc.compile()
    return nc


def make_in_maps(x, w_qkv, b_qkv, w_proj, b_proj, N=1024, D=1024, H=16, DK=64):
    """Host-side prep: shard over batch, fold scale, transpose x, cast bf16."""
    bf = ml_dtypes.bfloat16
    KC = D // P
    scale = np.float32(1.0 / np.sqrt(DK))
    wq = np.ascontiguousarray((w_qkv[:, :D] * scale)).astype(bf)
    wk = np.ascontiguousarray(w_qkv[:, D : 2 * D]).astype(bf)
    wv = np.ascontiguousarray(w_qkv[:, 2 * D :]).astype(bf)
    wp = np.ascontiguousarray(w_proj).astype(bf)
    bq = np.ascontiguousarray((b_qkv[:D] * scale).reshape(KC, P).T).astype(np.float32)
    bk = np.ascontiguousarray(b_qkv[D : 2 * D].reshape(KC, P).T).astype(np.float32)
    bvb = np.ascontiguousarray(np.broadcast_to(b_qkv[2 * D :], (P, D))).astype(bf)
    bp = np.ascontiguousarray(b_proj.reshape(KC, P).T).astype(np.float32)
    in_maps = []
    for b in range(x.shape[0]):
        xT = np.ascontiguousarray(x[b].T).astype(bf)
        in_maps.append(
            dict(xT=xT, wq=wq, wk=wk, wv=wv, wp=wp, bq=bq, bk=bk, bvb=bvb, bp=bp)
        )
    return in_maps


_module_cache = {}


def kernel(x, w_qkv, b_qkv, w_proj, b_proj):
    from concourse.bass_utils import run_bass_kernel_spmd

    x = np.asarray(x)
    B = x.shape[0]
    if "nc" not in _module_cache:
        _module_cache["nc"] = build_module()
    nc = _module_cache["nc"]
    in_maps = make_in_maps(
        x, np.asarray(w_qkv), np.asarray(b_qkv), np.asarray(w_proj), np.asarray(b_proj)
    )
    res = run_bass_kernel_spmd(nc, in_maps, core_ids=list(range(B)))
    out = np.stack([np.asarray(r["yT"]).T for r in res.results], axis=0)
    return np.ascontiguousarray(out.astype(np.float32))
